# revision 15
# baseline (speedup 1.0000x reference)
"""Trainium2 Bass kernel for nn_CSPN (sum-product network layer).

out[b,s] = logsumexp_n(prod[b,n] + log_softmax_n(gate[b,n,s]))
         = log(S1[b,s]) - log(S0[b,s]) + C1[b]
where prod[b,n] = lp1[b, n%32] + lp2[b, n//32]  (Gaussian leaf log-probs),
      C1[b] = max_n prod[b,n]  (shift needed: exp(prod) ~ e^-90 underflows),
      S1 = sum_n exp(prod - C1) * exp(g),  S0 = sum_n exp(g).

Sharding: data-parallel over batch, 512 rows per core, no communication.

Per-core dataflow: batch on partitions for the (cheap) leaf prep; the big
n=1024 reduction runs on TensorE as accumulating matmuls with n spread over
partitions as n = 8p + i (octet-of-n per partition). That mapping keeps the
gate DMA 3-dim with 1KB contiguous DRAM runs (~2.5x the bandwidth of the
naive n-on-partitions layout, which degrades to 128B runs). For each i in
0..8, stationary = [16 w-columns | ones] (bf16), moving = exp(gate) (bf16,
strided slice), contracting 128 octets at a time into fp32 PSUM [32, 512]:
row m<16 = S1 for batch row m, rows 16+ = S0. Ln with per-partition scale
exp(C1+DELTA) folds the shift back in; a 32x32 DVE transpose + strided
diagonal extraction yields out[b, s].
"""

import sys

sys.path.insert(0, "/opt/trn_rl_repo")

import numpy as np

B = 4096
K = 32          # gaussians per region
S = 32          # gating outputs
N = K * K       # 1024 products
NCORES = 8
BC = B // NCORES    # 512 batch rows per core
P = 128
NT = BC // P        # 4 batch tiles per core
NG = P // 16        # 8 groups of 16 batch rows per tile
NI = N // P         # 8 n's per partition (octet)

LOG2PI = float(np.log(2.0 * np.pi))
BCONST = -K * LOG2PI    # prod = -0.5*(raw1+raw2) + BCONST
DELTA = 85.0            # centers Ln inputs in the ACT Ln LUT domain [e^-44, e^44]

_cache = {}


def _strided_cols(bass, ap, start, step, count):
    """AP selecting free columns start, start+step, ... of a [P, F] AP."""
    return bass.AP(
        tensor=ap.tensor,
        offset=ap.offset + start * ap.ap[-1][0],
        ap=[ap.ap[0], [step * ap.ap[-1][0], count]],
    )


def _patch_act_tables():
    """Make the table-load chooser use the combined exp+ln set so Exp and Ln
    activations don't ping-pong ~1.3us ACT_TABLE_LOADs between two sets.
    Set ids are positional, so contents are masked rather than reordered."""
    from concourse import bacc, hw_specs
    import concourse.mybir as mybir

    if getattr(bacc, "_act_tables_patched", False):
        return
    orig = hw_specs.get_activation_tables

    def patched(module_arch):
        tabs = orig(module_arch)
        AF = mybir.ActivationFunctionType
        both = {AF.Exp, AF.Ln}
        return {
            name: (fns - both if name != "natural_log_exp_and_others" else fns)
            for name, fns in tabs.items()
        }

    bacc.get_activation_tables = patched
    bacc._act_tables_patched = True


def _build():
    import concourse.bass as bass
    import concourse.mybir as mybir
    import concourse.tile as tile
    from concourse import bacc
    from concourse.masks import make_identity

    _patch_act_tables()

    fp32 = mybir.dt.float32
    bf16 = mybir.dt.bfloat16
    AF = mybir.ActivationFunctionType
    ALU = mybir.AluOpType
    AX = mybir.AxisListType

    nc = bacc.Bacc("TRN2", target_bir_lowering=False, debug=False)
    x1 = nc.declare_dram_parameter("x1", [BC, K], fp32, isOutput=False)
    x2 = nc.declare_dram_parameter("x2", [BC, K], fp32, isOutput=False)
    m1 = nc.declare_dram_parameter("m1", [BC, N], fp32, isOutput=False)
    m2 = nc.declare_dram_parameter("m2", [BC, N], fp32, isOutput=False)
    g = nc.declare_dram_parameter("g", [BC, N * S], fp32, isOutput=False)
    out = nc.declare_dram_parameter("out", [BC, S], fp32, isOutput=True)

    # [b, p, c]: p = n-octet on partitions, c = i*32 + s (1KB contiguous)
    g3 = g.rearrange("b (p c) -> b p c", p=P)
    out3 = out.rearrange("(k p) s -> k p s", p=K)  # [16, 32, 32]

    with (
        tile.TileContext(nc) as tc,
        tc.tile_pool(name="const", bufs=1) as constp,
        tc.tile_pool(name="prep", bufs=2) as prep,
        tc.tile_pool(name="wpool", bufs=NT * NI) as wpool,
        tc.tile_pool(name="gate", bufs=4) as gatep,
        tc.tile_pool(name="eg", bufs=4) as egp,
        tc.tile_pool(name="small", bufs=4) as small,
        tc.tile_pool(name="psum_t", bufs=2, space="PSUM") as psum_t,
        tc.tile_pool(name="psum_o", bufs=4, space="PSUM") as psum_o,
        tc.tile_pool(name="dram", bufs=2, space="DRAM") as dramp,
    ):
        ident = constp.tile([P, P], fp32)
        make_identity(nc, ident[:])
        outT = constp.tile([K, BC], fp32)  # [32 s, 512 b]
        bconst = constp.tile([P, 1], fp32)
        nc.vector.memset(bconst[:], BCONST + DELTA)

        # ---- phase A: all prep up front (keeps in-order engine queues
        # free of mid-stream stalls during the main gate streaming loop) ----
        xts, mts = [], []
        for t in range(NT):
            bs = t * P
            xt1 = prep.tile([P, K], fp32, tag="x1", bufs=NT, name=f"xt1_{t}")
            nc.sync.dma_start(out=xt1[:], in_=x1[bs : bs + P, :])
            xt2 = prep.tile([P, K], fp32, tag="x2", bufs=NT, name=f"xt2_{t}")
            nc.sync.dma_start(out=xt2[:], in_=x2[bs : bs + P, :])
            mt1 = prep.tile([P, K, K], fp32, tag="m1", bufs=NT, name=f"mt1_{t}")
            nc.scalar.dma_start(
                out=mt1[:], in_=m1[bs : bs + P, :].rearrange("p (i j) -> p i j", j=K)
            )
            mt2 = prep.tile([P, K, K], fp32, tag="m2", bufs=NT, name=f"mt2_{t}")
            nc.scalar.dma_start(
                out=mt2[:], in_=m2[bs : bs + P, :].rearrange("p (i j) -> p i j", j=K)
            )
            xts.append((xt1, xt2))
            mts.append((mt1, mt2))

        w_all, scale_all = [], []
        for t in range(NT):
            raws = []
            minrs = []
            for xt, mt, tagn in (
                (xts[t][0], mts[t][0], "1"),
                (xts[t][1], mts[t][1], "2"),
            ):
                d = prep.tile([P, K, K], fp32, tag="d" + tagn, name=f"d{tagn}_{t}")
                nc.vector.tensor_sub(
                    d[:], mt[:], xt[:].unsqueeze(1).broadcast_to([P, K, K])
                )
                nc.vector.tensor_mul(d[:], d[:], d[:])
                raw = prep.tile([P, K], fp32, tag="raw" + tagn, name=f"raw{tagn}_{t}")
                nc.vector.tensor_reduce(raw[:], d[:], axis=AX.X, op=ALU.add)
                minr = prep.tile([P, 1], fp32, tag="minr" + tagn, name=f"minr{tagn}_{t}")
                nc.vector.tensor_reduce(minr[:], raw[:], axis=AX.X, op=ALU.min)
                nc.vector.tensor_sub(raw[:], raw[:], minr[:].broadcast_to([P, K]))
                raws.append(raw)
                minrs.append(minr)

            rawp = prep.tile([P, K, K], fp32, tag="rawp", name=f"rawp_{t}")
            nc.vector.tensor_add(
                rawp[:],
                raws[1][:].unsqueeze(2).broadcast_to([P, K, K]),
                raws[0][:].unsqueeze(1).broadcast_to([P, K, K]),
            )
            rawp_f = rawp[:].rearrange("p i j -> p (i j)")

            # scale vector exp(C1 + DELTA), reshaped [16 m, 8 g] (+pad rows)
            c1 = prep.tile([P, 1], fp32, tag="c1", name=f"c1_{t}")
            nc.vector.tensor_add(c1[:], minrs[0][:], minrs[1][:])
            expc = prep.tile([P, 1], fp32, tag="expc", name=f"expc_{t}")
            nc.scalar.activation(
                expc[:], c1[:], AF.Exp, bias=bconst[:], scale=-0.5
            )
            dscr = dramp.tile([1, P], fp32, tag="dscr", name=f"dscr_{t}")
            nc.sync.dma_start(out=dscr[:], in_=expc[:])
            scale_t = prep.tile([K, NG], fp32, tag="scale", bufs=NT, name=f"scale_{t}")
            nc.vector.memset(scale_t[:], 1.0)
            nc.sync.dma_start(
                out=scale_t[0:16, :],
                in_=dscr[:].rearrange("o (g m) -> o m g", m=16),
            )
            scale_all.append(scale_t)

            # stationary weights: w[i][p, g, m] = exp(-0.5*raw'[b=16g+m, 8p+i])
            # columns 16..31 of each group = 1.0 (S0 ones col; M=32 pad)
            w_tiles = []
            for i in range(NI):
                pt = psum_t.tile([P, P], fp32, tag="pt", name=f"pt_{t}_{i}")
                nc.tensor.transpose(
                    pt[:], _strided_cols(bass, rawp_f, i, NI, P), ident[:]
                )
                w_t = wpool.tile([P, NG, K], bf16, tag="w", name=f"w_{t}_{i}")
                nc.vector.memset(w_t[:], 1.0)
                nc.scalar.activation(
                    w_t[:, :, 0:16],
                    pt[:].rearrange("p (g m) -> p g m", m=16),
                    AF.Exp,
                    scale=-0.5,
                )
                w_tiles.append(w_t)
            w_all.append(w_tiles)

        # ---- phase B: stream gates (1KB runs), exp->bf16, matmul over n ----
        for t in range(NT):
            bs = t * P
            w_tiles = w_all[t]
            scale_t = scale_all[t]
            for q in range(NG // 2):  # quarters of 32 batch rows
                pos = [
                    psum_o.tile([K, 512], fp32, tag="po", name=f"po_{t}_{q}_{h}")
                    for h in range(2)
                ]
                egs = []
                for h in range(2):
                    bh = bs + q * 32 + h * 16
                    gt = gatep.tile([P, 16, NI, S], fp32, tag="gt", name=f"gt_{t}_{q}_{h}")
                    eng = nc.sync if h == 0 else nc.gpsimd
                    eng.dma_start(
                        out=gt[:],
                        in_=g3[bh : bh + 16, :, :].transpose([1, 0, 2]),
                    )
                    eg = egp.tile([P, 16, NI, S], bf16, tag="eg", name=f"eg_{t}_{q}_{h}")
                    nc.scalar.activation(eg[:], gt[:], AF.Exp)
                    egs.append(eg)
                for i in range(NI):
                    for h in range(2):
                        nc.tensor.matmul(
                            pos[h][:],
                            w_tiles[i][:, 2 * q + h, :],
                            egs[h][:, :, i, :],
                            start=(i == 0),
                            stop=(i == NI - 1),
                        )
                for h in range(2):
                    gi = 2 * q + h
                    bg = bs + gi * 16
                    lg = small.tile([K, 512], fp32, tag="lg", name=f"lg_{t}_{q}_{h}")
                    nc.scalar.activation(
                        lg[:], pos[h][:], AF.Ln, scale=scale_t[:, gi : gi + 1]
                    )
                    T = small.tile([K, 512], fp32, tag="T", name=f"T_{t}_{q}_{h}")
                    nc.vector.transpose(T[:], lg[:])
                    nc.vector.scalar_tensor_tensor(
                        out=outT[:, bg : bg + 16],
                        in0=_strided_cols(bass, T[:], 0, 33, 16),
                        scalar=DELTA,
                        in1=_strided_cols(bass, T[:], 16, 32, 16),
                        op0=mybir.AluOpType.subtract,
                        op1=mybir.AluOpType.subtract,
                    )

        Gt = constp.tile([K, BC], fp32)
        nc.vector.transpose(Gt[:], outT[:])
        nc.sync.dma_start(
            out=out3.transpose([1, 0, 2]),
            in_=Gt[:].rearrange("p (k s) -> p k s", s=S),
        )

    nc.compile()
    return nc


def _get_nc():
    if "nc" not in _cache:
        _cache["nc"] = _build()
    return _cache["nc"]


def kernel(x, means1, means2, gate_params, scope1, scope2):
    from concourse.bass_utils import run_bass_kernel_spmd

    x = np.asarray(x, dtype=np.float32)
    means1 = np.ascontiguousarray(np.asarray(means1, dtype=np.float32))
    means2 = np.ascontiguousarray(np.asarray(means2, dtype=np.float32))
    gp = np.ascontiguousarray(
        np.asarray(gate_params, dtype=np.float32).reshape(B, N * S)
    )
    xs1 = np.ascontiguousarray(x[:, np.asarray(scope1)])
    xs2 = np.ascontiguousarray(x[:, np.asarray(scope2)])

    nc = _get_nc()
    in_maps = []
    for c in range(NCORES):
        sl = slice(c * BC, (c + 1) * BC)
        in_maps.append(
            {
                "x1": xs1[sl],
                "x2": xs2[sl],
                "m1": means1[sl],
                "m2": means2[sl],
                "g": gp[sl],
            }
        )
    res = run_bass_kernel_spmd(nc, in_maps, core_ids=list(range(NCORES)))
    return np.concatenate([res.results[c]["out"] for c in range(NCORES)], axis=0)


# revision 16
# speedup vs baseline: 1.1568x; 1.1568x over previous
"""Trainium2 Bass kernel for nn_CSPN (sum-product network layer).

out[b,s] = logsumexp_n(prod[b,n] + log_softmax_n(gate[b,n,s]))
         = log(S1[b,s]) - log(S0[b,s]) + C1[b]
where prod[b,n] = lp1[b, n%32] + lp2[b, n//32]  (Gaussian leaf log-probs),
      C1[b] = max_n prod[b,n]  (shift needed: exp(prod) ~ e^-90 underflows),
      S1 = sum_n exp(prod - C1) * exp(g),  S0 = sum_n exp(g).

Sharding: data-parallel over batch, 512 rows per core, no communication.

Per-core dataflow: batch on partitions for the (cheap) leaf prep; the big
n=1024 reduction runs on TensorE as accumulating matmuls with n spread over
partitions as n = 8p + i (octet-of-n per partition). That mapping keeps the
gate DMA 3-dim with 1KB contiguous DRAM runs (~2.5x the bandwidth of the
naive n-on-partitions layout, which degrades to 128B runs). For each i in
0..8, stationary = [16 w-columns | ones] (bf16), moving = exp(gate) (bf16,
strided slice), contracting 128 octets at a time into fp32 PSUM [32, 512]:
row m<16 = S1 for batch row m, rows 16+ = S0. Ln with per-partition scale
exp(C1+DELTA) folds the shift back in; a 32x32 DVE transpose + strided
diagonal extraction yields out[b, s].
"""

import sys

sys.path.insert(0, "/opt/trn_rl_repo")

import numpy as np

B = 4096
K = 32          # gaussians per region
S = 32          # gating outputs
N = K * K       # 1024 products
NCORES = 8
BC = B // NCORES    # 512 batch rows per core
P = 128
NT = BC // P        # 4 batch tiles per core
NG = P // 16        # 8 groups of 16 batch rows per tile
NI = N // P         # 8 n's per partition (octet)

LOG2PI = float(np.log(2.0 * np.pi))
BCONST = -K * LOG2PI    # prod = -0.5*(raw1+raw2) + BCONST
DELTA = 85.0            # centers Ln inputs in the ACT Ln LUT domain [e^-44, e^44]

_cache = {}


def _strided_cols(bass, ap, start, step, count):
    """AP selecting free columns start, start+step, ... of a [P, F] AP."""
    return bass.AP(
        tensor=ap.tensor,
        offset=ap.offset + start * ap.ap[-1][0],
        ap=[ap.ap[0], [step * ap.ap[-1][0], count]],
    )


def _patch_act_tables():
    """Make the table-load chooser use the combined exp+ln set so Exp and Ln
    activations don't ping-pong ~1.3us ACT_TABLE_LOADs between two sets.
    Set ids are positional, so contents are masked rather than reordered."""
    from concourse import bacc, hw_specs
    import concourse.mybir as mybir

    if getattr(bacc, "_act_tables_patched", False):
        return
    orig = hw_specs.get_activation_tables

    def patched(module_arch):
        tabs = orig(module_arch)
        AF = mybir.ActivationFunctionType
        both = {AF.Exp, AF.Ln}
        return {
            name: (fns - both if name != "natural_log_exp_and_others" else fns)
            for name, fns in tabs.items()
        }

    bacc.get_activation_tables = patched
    bacc._act_tables_patched = True


def _build():
    import concourse.bass as bass
    import concourse.mybir as mybir
    import concourse.tile as tile
    from concourse import bacc
    from concourse.masks import make_identity

    _patch_act_tables()

    fp32 = mybir.dt.float32
    bf16 = mybir.dt.bfloat16
    AF = mybir.ActivationFunctionType
    ALU = mybir.AluOpType
    AX = mybir.AxisListType

    nc = bacc.Bacc("TRN2", target_bir_lowering=False, debug=False)
    x1 = nc.declare_dram_parameter("x1", [BC, K], fp32, isOutput=False)
    x2 = nc.declare_dram_parameter("x2", [BC, K], fp32, isOutput=False)
    m1 = nc.declare_dram_parameter("m1", [BC, N], fp32, isOutput=False)
    m2 = nc.declare_dram_parameter("m2", [BC, N], fp32, isOutput=False)
    g = nc.declare_dram_parameter("g", [BC, N * S], fp32, isOutput=False)
    out = nc.declare_dram_parameter("out", [BC, S], fp32, isOutput=True)

    # [b, p, c]: p = n-octet on partitions, c = i*32 + s (1KB contiguous)
    g3 = g.rearrange("b (p c) -> b p c", p=P)
    out3 = out.rearrange("(k p) s -> k p s", p=K)  # [16, 32, 32]

    with (
        tile.TileContext(nc) as tc,
        tc.tile_pool(name="const", bufs=1) as constp,
        tc.tile_pool(name="prep", bufs=2) as prep,
        tc.tile_pool(name="wpool", bufs=NT * NI) as wpool,
        tc.tile_pool(name="gate", bufs=4) as gatep,
        tc.tile_pool(name="eg", bufs=4) as egp,
        tc.tile_pool(name="small", bufs=4) as small,
        tc.tile_pool(name="psum_t", bufs=2, space="PSUM") as psum_t,
        tc.tile_pool(name="psum_o", bufs=4, space="PSUM") as psum_o,
        tc.tile_pool(name="dram", bufs=2, space="DRAM") as dramp,
    ):
        ident = constp.tile([P, P], fp32)
        make_identity(nc, ident[:])
        outT = constp.tile([K, BC], fp32)  # [32 s, 512 b]
        bconst = constp.tile([P, 1], fp32)
        nc.vector.memset(bconst[:], BCONST + DELTA)

        # Prep for tile t is emitted just-in-time (one tile ahead of the
        # streaming loop) so in-order engine queues never head-block the
        # gate exp/matmul stream behind long prep chains.
        w_all = {}
        scale_all = {}

        def emit_prep(t):
            bs = t * P
            xt1 = prep.tile([P, K], fp32, tag="x1", name=f"xt1_{t}")
            nc.sync.dma_start(out=xt1[:], in_=x1[bs : bs + P, :])
            xt2 = prep.tile([P, K], fp32, tag="x2", name=f"xt2_{t}")
            nc.sync.dma_start(out=xt2[:], in_=x2[bs : bs + P, :])
            mt1 = prep.tile([P, K, K], fp32, tag="m1", name=f"mt1_{t}")
            nc.sync.dma_start(
                out=mt1[:], in_=m1[bs : bs + P, :].rearrange("p (i j) -> p i j", j=K)
            )
            mt2 = prep.tile([P, K, K], fp32, tag="m2", name=f"mt2_{t}")
            nc.sync.dma_start(
                out=mt2[:], in_=m2[bs : bs + P, :].rearrange("p (i j) -> p i j", j=K)
            )

            raws = []
            minrs = []
            for xt, mt, tagn in ((xt1, mt1, "1"), (xt2, mt2, "2")):
                d = prep.tile([P, K, K], fp32, tag="d" + tagn, name=f"d{tagn}_{t}")
                nc.vector.tensor_sub(
                    d[:], mt[:], xt[:].unsqueeze(1).broadcast_to([P, K, K])
                )
                nc.vector.tensor_mul(d[:], d[:], d[:])
                raw = prep.tile([P, K], fp32, tag="raw" + tagn, name=f"raw{tagn}_{t}")
                nc.vector.tensor_reduce(raw[:], d[:], axis=AX.X, op=ALU.add)
                minr = prep.tile([P, 1], fp32, tag="minr" + tagn, name=f"minr{tagn}_{t}")
                nc.vector.tensor_reduce(minr[:], raw[:], axis=AX.X, op=ALU.min)
                nc.vector.tensor_sub(raw[:], raw[:], minr[:].broadcast_to([P, K]))
                raws.append(raw)
                minrs.append(minr)

            rawp = prep.tile([P, K, K], fp32, tag="rawp", name=f"rawp_{t}")
            nc.vector.tensor_add(
                rawp[:],
                raws[1][:].unsqueeze(2).broadcast_to([P, K, K]),
                raws[0][:].unsqueeze(1).broadcast_to([P, K, K]),
            )
            rawp_f = rawp[:].rearrange("p i j -> p (i j)")

            # scale vector exp(C1 + DELTA), reshaped [16 m, 8 g] (+pad rows)
            c1 = prep.tile([P, 1], fp32, tag="c1", name=f"c1_{t}")
            nc.vector.tensor_add(c1[:], minrs[0][:], minrs[1][:])
            expc = prep.tile([P, 1], fp32, tag="expc", name=f"expc_{t}")
            nc.scalar.activation(
                expc[:], c1[:], AF.Exp, bias=bconst[:], scale=-0.5
            )
            dscr = dramp.tile([1, P], fp32, tag="dscr", name=f"dscr_{t}")
            nc.sync.dma_start(out=dscr[:], in_=expc[:])
            scale_t = prep.tile([K, NG], fp32, tag="scale", bufs=NT, name=f"scale_{t}")
            nc.vector.memset(scale_t[:], 1.0)
            nc.sync.dma_start(
                out=scale_t[0:16, :],
                in_=dscr[:].rearrange("o (g m) -> o m g", m=16),
            )
            scale_all[t] = scale_t

            # stationary weights: w_t[p, i, g, m] = exp(-0.5*raw'[b=16g+m, 8p+i])
            # columns 16..31 of each group = 1.0 (S0 ones col; M=32 pad)
            w_t = wpool.tile([P, NI, NG, K], bf16, tag="w", bufs=2, name=f"w_{t}")
            nc.vector.memset(w_t[:], 1.0)
            for hb in range(2):
                pt = psum_t.tile([P, 4 * P], fp32, tag="pt", name=f"pt_{t}_{hb}")
                for ii in range(4):
                    i = 4 * hb + ii
                    nc.tensor.transpose(
                        pt[:, ii * P : (ii + 1) * P],
                        _strided_cols(bass, rawp_f, i, NI, P),
                        ident[:],
                    )
                nc.scalar.activation(
                    w_t[:, 4 * hb : 4 * hb + 4, :, 0:16],
                    pt[:].rearrange("p (ii g m) -> p ii g m", ii=4, m=16),
                    AF.Exp,
                    scale=-0.5,
                )
            w_all[t] = w_t

        def emit_quarter(t, q):
            bs = t * P
            w_t = w_all[t]
            scale_t = scale_all[t]
            pos = [
                psum_o.tile([K, 512], fp32, tag="po", name=f"po_{t}_{q}_{h}")
                for h in range(2)
            ]
            egs = []
            for h in range(2):
                bh = bs + q * 32 + h * 16
                gt = gatep.tile([P, 16, NI, S], fp32, tag="gt", name=f"gt_{t}_{q}_{h}")
                eng = nc.sync if h == 0 else nc.gpsimd
                eng.dma_start(
                    out=gt[:],
                    in_=g3[bh : bh + 16, :, :].transpose([1, 0, 2]),
                )
                eg = egp.tile([P, 16, NI, S], bf16, tag="eg", name=f"eg_{t}_{q}_{h}")
                nc.scalar.activation(eg[:], gt[:], AF.Exp)
                egs.append(eg)
            for i in range(NI):
                for h in range(2):
                    nc.tensor.matmul(
                        pos[h][:],
                        w_t[:, i, 2 * q + h, :],
                        egs[h][:, :, i, :],
                        start=(i == 0),
                        stop=(i == NI - 1),
                    )
            for h in range(2):
                gi = 2 * q + h
                bg = bs + gi * 16
                lg = small.tile([K, 512], fp32, tag="lg", name=f"lg_{t}_{q}_{h}")
                nc.scalar.activation(
                    lg[:], pos[h][:], AF.Ln, scale=scale_t[:, gi : gi + 1]
                )
                T = small.tile([K, 512], fp32, tag="T", name=f"T_{t}_{q}_{h}")
                nc.vector.transpose(T[:], lg[:])
                nc.vector.scalar_tensor_tensor(
                    out=outT[:, bg : bg + 16],
                    in0=_strided_cols(bass, T[:], 0, 33, 16),
                    scalar=DELTA,
                    in1=_strided_cols(bass, T[:], 16, 32, 16),
                    op0=mybir.AluOpType.subtract,
                    op1=mybir.AluOpType.subtract,
                )

        NQ = NG // 2
        emit_prep(0)
        for step in range(NT * NQ):
            t, q = divmod(step, NQ)
            if q == 1 and t + 1 < NT:
                emit_prep(t + 1)
            emit_quarter(t, q)

        Gt = constp.tile([K, BC], fp32)
        nc.vector.transpose(Gt[:], outT[:])
        nc.sync.dma_start(
            out=out3.transpose([1, 0, 2]),
            in_=Gt[:].rearrange("p (k s) -> p k s", s=S),
        )

    nc.compile()
    return nc


def _get_nc():
    if "nc" not in _cache:
        _cache["nc"] = _build()
    return _cache["nc"]


def kernel(x, means1, means2, gate_params, scope1, scope2):
    from concourse.bass_utils import run_bass_kernel_spmd

    x = np.asarray(x, dtype=np.float32)
    means1 = np.ascontiguousarray(np.asarray(means1, dtype=np.float32))
    means2 = np.ascontiguousarray(np.asarray(means2, dtype=np.float32))
    gp = np.ascontiguousarray(
        np.asarray(gate_params, dtype=np.float32).reshape(B, N * S)
    )
    xs1 = np.ascontiguousarray(x[:, np.asarray(scope1)])
    xs2 = np.ascontiguousarray(x[:, np.asarray(scope2)])

    nc = _get_nc()
    in_maps = []
    for c in range(NCORES):
        sl = slice(c * BC, (c + 1) * BC)
        in_maps.append(
            {
                "x1": xs1[sl],
                "x2": xs2[sl],
                "m1": means1[sl],
                "m2": means2[sl],
                "g": gp[sl],
            }
        )
    res = run_bass_kernel_spmd(nc, in_maps, core_ids=list(range(NCORES)))
    return np.concatenate([res.results[c]["out"] for c in range(NCORES)], axis=0)


# revision 20
# speedup vs baseline: 1.2222x; 1.0566x over previous
"""Trainium2 Bass kernel for nn_CSPN (sum-product network layer).

out[b,s] = logsumexp_n(prod[b,n] + log_softmax_n(gate[b,n,s]))
         = log(S1[b,s]) - log(S0[b,s]) + C1[b]
where prod[b,n] = lp1[b, n%32] + lp2[b, n//32]  (Gaussian leaf log-probs),
      C1[b] = max_n prod[b,n]  (shift needed: exp(prod) ~ e^-90 underflows),
      S1 = sum_n exp(prod - C1) * exp(g),  S0 = sum_n exp(g).

Sharding: data-parallel over batch, 512 rows per core, no communication.

Per-core dataflow: batch on partitions for the (cheap) leaf prep; the big
n=1024 reduction runs on TensorE as accumulating matmuls with n spread over
partitions as n = 8p + i (octet-of-n per partition). That mapping keeps the
gate DMA 3-dim with 1KB contiguous DRAM runs (~2.5x the bandwidth of the
naive n-on-partitions layout, which degrades to 128B runs). For each i in
0..8, stationary = [16 w-columns | ones] (bf16), moving = exp(gate) (bf16,
strided slice), contracting 128 octets at a time into fp32 PSUM [32, 512]:
row m<16 = S1 for batch row m, rows 16+ = S0. Ln with per-partition scale
exp(C1+DELTA) folds the shift back in; a 32x32 DVE transpose + strided
diagonal extraction yields out[b, s].
"""

import sys

sys.path.insert(0, "/opt/trn_rl_repo")

import numpy as np

B = 4096
K = 32          # gaussians per region
S = 32          # gating outputs
N = K * K       # 1024 products
NCORES = 8
BC = B // NCORES    # 512 batch rows per core
P = 128
NT = BC // P        # 4 batch tiles per core
NG = P // 16        # 8 groups of 16 batch rows per tile
NI = N // P         # 8 n's per partition (octet)

LOG2PI = float(np.log(2.0 * np.pi))
BCONST = -K * LOG2PI    # prod = -0.5*(raw1+raw2) + BCONST
DELTA = 85.0            # centers Ln inputs in the ACT Ln LUT domain [e^-44, e^44]

_cache = {}


def _strided_cols(bass, ap, start, step, count):
    """AP selecting free columns start, start+step, ... of a [P, F] AP."""
    return bass.AP(
        tensor=ap.tensor,
        offset=ap.offset + start * ap.ap[-1][0],
        ap=[ap.ap[0], [step * ap.ap[-1][0], count]],
    )


def _patch_act_tables():
    """Make the table-load chooser use the combined exp+ln set so Exp and Ln
    activations don't ping-pong ~1.3us ACT_TABLE_LOADs between two sets.
    Set ids are positional, so contents are masked rather than reordered."""
    from concourse import bacc, hw_specs
    import concourse.mybir as mybir

    if getattr(bacc, "_act_tables_patched", False):
        return
    orig = hw_specs.get_activation_tables

    def patched(module_arch):
        tabs = orig(module_arch)
        AF = mybir.ActivationFunctionType
        both = {AF.Exp, AF.Ln}
        return {
            name: (fns - both if name != "natural_log_exp_and_others" else fns)
            for name, fns in tabs.items()
        }

    bacc.get_activation_tables = patched
    bacc._act_tables_patched = True


def _build():
    import concourse.bass as bass
    import concourse.mybir as mybir
    import concourse.tile as tile
    from concourse import bacc
    from concourse.masks import make_identity

    _patch_act_tables()

    fp32 = mybir.dt.float32
    bf16 = mybir.dt.bfloat16
    AF = mybir.ActivationFunctionType
    ALU = mybir.AluOpType
    AX = mybir.AxisListType

    nc = bacc.Bacc("TRN2", target_bir_lowering=False, debug=False)
    x1 = nc.declare_dram_parameter("x1", [BC, K], fp32, isOutput=False)
    x2 = nc.declare_dram_parameter("x2", [BC, K], fp32, isOutput=False)
    m1 = nc.declare_dram_parameter("m1", [BC, N], fp32, isOutput=False)
    m2 = nc.declare_dram_parameter("m2", [BC, N], fp32, isOutput=False)
    g = nc.declare_dram_parameter("g", [BC, N * S], fp32, isOutput=False)
    out = nc.declare_dram_parameter("out", [BC, S], fp32, isOutput=True)

    # [b, p, c]: p = n-octet on partitions, c = i*32 + s (1KB contiguous)
    g3 = g.rearrange("b (p c) -> b p c", p=P)
    out3 = out.rearrange("(k p) s -> k p s", p=K)  # [16, 32, 32]

    with (
        tile.TileContext(nc) as tc,
        tc.tile_pool(name="const", bufs=1) as constp,
        tc.tile_pool(name="prep", bufs=2) as prep,
        tc.tile_pool(name="wpool", bufs=NT * NI) as wpool,
        tc.tile_pool(name="gate", bufs=3) as gatep,
        tc.tile_pool(name="eg", bufs=2) as egp,
        tc.tile_pool(name="small", bufs=2) as small,
        tc.tile_pool(name="psum_t", bufs=2, space="PSUM") as psum_t,
        tc.tile_pool(name="psum_o", bufs=3, space="PSUM") as psum_o,
        tc.tile_pool(name="dram", bufs=2, space="DRAM") as dramp,
    ):
        ident = constp.tile([P, P], fp32)
        make_identity(nc, ident[:])
        outT = constp.tile([K, BC], fp32)  # [32 s, 512 b]
        bconst = constp.tile([P, 1], fp32)
        nc.vector.memset(bconst[:], BCONST + DELTA)

        # Prep for tile t is emitted just-in-time (one tile ahead of the
        # streaming loop) so in-order engine queues never head-block the
        # gate exp/matmul stream behind long prep chains.
        w_all = {}
        c1d = dramp.tile([1, BC], fp32, tag="c1d", name="c1d")

        def emit_prep(t):
            bs = t * P
            xt1 = prep.tile([P, K], fp32, tag="x1", name=f"xt1_{t}")
            nc.sync.dma_start(out=xt1[:], in_=x1[bs : bs + P, :])
            xt2 = prep.tile([P, K], fp32, tag="x2", name=f"xt2_{t}")
            nc.sync.dma_start(out=xt2[:], in_=x2[bs : bs + P, :])
            mt1 = prep.tile([P, K, K], fp32, tag="m1", name=f"mt1_{t}")
            nc.sync.dma_start(
                out=mt1[:], in_=m1[bs : bs + P, :].rearrange("p (i j) -> p i j", j=K)
            )
            mt2 = prep.tile([P, K, K], fp32, tag="m2", name=f"mt2_{t}")
            nc.sync.dma_start(
                out=mt2[:], in_=m2[bs : bs + P, :].rearrange("p (i j) -> p i j", j=K)
            )

            raws = []
            minrs = []
            for xt, mt, tagn in ((xt1, mt1, "1"), (xt2, mt2, "2")):
                d = prep.tile([P, K, K], fp32, tag="d" + tagn, name=f"d{tagn}_{t}")
                nc.vector.tensor_sub(
                    d[:], mt[:], xt[:].unsqueeze(1).broadcast_to([P, K, K])
                )
                nc.vector.tensor_mul(d[:], d[:], d[:])
                raw = prep.tile([P, K], fp32, tag="raw" + tagn, name=f"raw{tagn}_{t}")
                nc.vector.tensor_reduce(raw[:], d[:], axis=AX.X, op=ALU.add)
                minr = prep.tile([P, 1], fp32, tag="minr" + tagn, name=f"minr{tagn}_{t}")
                nc.vector.tensor_reduce(minr[:], raw[:], axis=AX.X, op=ALU.min)
                nc.vector.tensor_sub(raw[:], raw[:], minr[:].broadcast_to([P, K]))
                raws.append(raw)
                minrs.append(minr)

            rawp = prep.tile([P, K, K], fp32, tag="rawp", name=f"rawp_{t}")
            nc.vector.tensor_add(
                rawp[:],
                raws[1][:].unsqueeze(2).broadcast_to([P, K, K]),
                raws[0][:].unsqueeze(1).broadcast_to([P, K, K]),
            )
            rawp_f = rawp[:].rearrange("p i j -> p (i j)")

            # C1[b] = -0.5*(minr1+minr2) + BCONST, staged to DRAM for the
            # final partition-broadcast add.
            c1 = prep.tile([P, 1], fp32, tag="c1", name=f"c1_{t}")
            nc.vector.tensor_add(c1[:], minrs[0][:], minrs[1][:])
            nc.vector.tensor_scalar(
                out=c1[:], in0=c1[:], scalar1=-0.5, scalar2=BCONST,
                op0=ALU.mult, op1=ALU.add,
            )
            nc.sync.dma_start(out=c1d[0:1, bs : bs + P], in_=c1[:])

            # stationary weights: w_t[p, i, g, m] = exp(-0.5*raw'[b=16g+m, 8p+i])
            # columns 16..31 of each group = 1.0 (S0 ones col; M=32 pad)
            w_t = wpool.tile([P, NI, NG, K], bf16, tag="w", bufs=2, name=f"w_{t}")
            nc.vector.memset(w_t[:], 1.0)
            for hb in range(2):
                pt = psum_t.tile([P, 4 * P], fp32, tag="pt", name=f"pt_{t}_{hb}")
                for ii in range(4):
                    i = 4 * hb + ii
                    nc.tensor.transpose(
                        pt[:, ii * P : (ii + 1) * P],
                        _strided_cols(bass, rawp_f, i, NI, P),
                        ident[:],
                    )
                nc.scalar.activation(
                    w_t[:, 4 * hb : 4 * hb + 4, :, 0:16],
                    pt[:].rearrange("p (ii g m) -> p ii g m", ii=4, m=16),
                    AF.Exp,
                    scale=-0.5,
                )
            w_all[t] = w_t

        def emit_quarter(t, q):
            bs = t * P
            w_t = w_all[t]
            bq = bs + q * 32
            po = psum_o.tile([K, 2, 512], fp32, tag="po", name=f"po_{t}_{q}")
            gt = gatep.tile([P, 2, 16, NI, S], fp32, tag="gt", name=f"gt_{t}_{q}")
            for h in range(2):
                bh = bq + h * 16
                eng = nc.sync if h == 0 else nc.gpsimd
                eng.dma_start(
                    out=gt[:, h, :, :, :],
                    in_=g3[bh : bh + 16, :, :].transpose([1, 0, 2]),
                )
            eg = egp.tile([P, 2, 16, NI, S], bf16, tag="eg", name=f"eg_{t}_{q}")
            nc.scalar.activation(eg[:], gt[:], AF.Exp)
            for i in range(NI):
                for h in range(2):
                    nc.tensor.matmul(
                        po[:, h, :],
                        w_t[:, i, 2 * q + h, :],
                        eg[:, h, :, i, :],
                        start=(i == 0),
                        stop=(i == NI - 1),
                    )
            lg = small.tile([K, 2 * 512], fp32, tag="lg", name=f"lg_{t}_{q}")
            nc.scalar.activation(lg[:], po[:], AF.Ln)
            T = small.tile([K, 2 * 512], fp32, tag="T", name=f"T_{t}_{q}")
            nc.vector.transpose(T[:], lg[:])
            for h in range(2):
                bg = bq + h * 16
                nc.vector.tensor_sub(
                    outT[:, bg : bg + 16],
                    _strided_cols(bass, T[:], h * 512, 33, 16),
                    _strided_cols(bass, T[:], h * 512 + 16, 32, 16),
                )

        NQ = NG // 2
        emit_prep(0)
        for step in range(NT * NQ):
            t, q = divmod(step, NQ)
            if q == 1 and t + 1 < NT:
                emit_prep(t + 1)
            emit_quarter(t, q)

        # add C1[b]: broadcast [1, 512] DRAM row across the 32 s-partitions
        c1b = constp.tile([K, BC], fp32)
        nc.gpsimd.dma_start(
            out=c1b[:],
            in_=bass.AP(
                tensor=c1d[:].tensor,
                offset=c1d[:].offset,
                ap=[[0, K]] + [list(d) for d in c1d[:].ap[1:]],
            ),
        )
        nc.vector.tensor_add(outT[:], outT[:], c1b[:])

        Gt = constp.tile([K, BC], fp32)
        nc.vector.transpose(Gt[:], outT[:])
        nc.sync.dma_start(
            out=out3.transpose([1, 0, 2]),
            in_=Gt[:].rearrange("p (k s) -> p k s", s=S),
        )

    nc.compile()
    return nc


def _get_nc():
    if "nc" not in _cache:
        _cache["nc"] = _build()
    return _cache["nc"]


def kernel(x, means1, means2, gate_params, scope1, scope2):
    from concourse.bass_utils import run_bass_kernel_spmd

    x = np.asarray(x, dtype=np.float32)
    means1 = np.ascontiguousarray(np.asarray(means1, dtype=np.float32))
    means2 = np.ascontiguousarray(np.asarray(means2, dtype=np.float32))
    gp = np.ascontiguousarray(
        np.asarray(gate_params, dtype=np.float32).reshape(B, N * S)
    )
    xs1 = np.ascontiguousarray(x[:, np.asarray(scope1)])
    xs2 = np.ascontiguousarray(x[:, np.asarray(scope2)])

    nc = _get_nc()
    in_maps = []
    for c in range(NCORES):
        sl = slice(c * BC, (c + 1) * BC)
        in_maps.append(
            {
                "x1": xs1[sl],
                "x2": xs2[sl],
                "m1": means1[sl],
                "m2": means2[sl],
                "g": gp[sl],
            }
        )
    res = run_bass_kernel_spmd(nc, in_maps, core_ids=list(range(NCORES)))
    return np.concatenate([res.results[c]["out"] for c in range(NCORES)], axis=0)
